# revision 15
# baseline (speedup 1.0000x reference)
"""Multi-head attention (B=2, N=2048, C=1024, H=16) on 8 trn2 NeuronCores.

Tensor-parallel over heads: core c computes heads {2c, 2c+1} for both batch
elements and emits a partial output y_c = attn_out_c @ W_out[local rows];
the host sums the 8 partials (bf16) and adds b_out.

Per-core pipeline (single TileContext, fully unrolled):
  - Startup: warmup matmuls on the identity tile pre-warm the PE HAM clock
    gate while the first DMAs land.  The QKV weights (wall) and x window 0
    are dispatched FIRST, interleaved across all four DMA queues
    (sync/gpsimd/scalar/vector) so the first QKV matmul can start ~10us in
    (the baseline serialized x-then-wall on two queues; first MM at 18us).
  - QKV^T projection with stacked per-head weights; biases folded into the
    PSUM->SBUF eviction.
  - S^T = K @ Q^T with both local heads in full-K=128 matmuls via a
    zero-padded Q^T layout (K=64 matmuls measured ~2x slower than K=128).
  - P^T = exp(S^T / 32) on ScalarE straight from PSUM ([128, 1024] ops).
    The attention inner loop is ACT-bound: exp costs (1024+352)/1.2 =
    1147ns per k-chunk vs ~864ns of PE work, so all other PE work (QKV
    fills, output projections) is injected into the ACT shadow.
  - PV via ones-augmented V (65th stationary column accumulates the softmax
    denominators for free). V is transposed on the PE (128x128 tiles).
  - Normalization: one [64,512]+[1,512] bf16 eviction per head (releases
    the PV PSUM bank fast), ones-broadcast bf16 matmul of the sums row,
    reciprocal_approx_fast, DVE multiply -> out^T in bf16.
  - Output projection fully in bf16: fp32r matmuls measured HALF rate
    (fp32_mode=HIGH, ~426ns vs 216ns at free=512), so outT/wout are bf16.
  - y partials stored/DMAd as bf16 (halves the 16MB output traffic).

Emission order IS program order for Tile: batch-1 QKV projection units and
deferred projection stores are injected inside batch-0's attention loop
(after their producers).  Batch 1 has no QKV fills, so its loop gets a
double ration of deferred projection pops to keep the PE fed.  Never emit
a consumer before its producer: reads of not-yet-written SBUF regions
silently bind to stale contents.

Baseline measured 244.9us; this version targets ~195us (PE busy ~185us ->
~170us via bf16 projection; startup 18->10us; tail 14->8us).
absmax error ~5e-3 of output scale vs the fp32 reference (bf16 rounding).
"""
import sys

sys.path.insert(0, "/opt/trn_rl_repo")

import ml_dtypes
import numpy as np

import concourse.bacc as bacc
import concourse.mybir as mybir
import concourse.tile as tile
from concourse import bass_utils
from concourse.masks import make_identity

F32 = mybir.dt.float32
BF16 = mybir.dt.bfloat16
NPBF16 = ml_dtypes.bfloat16

EMB = 1024
HEADS = 16
B = 2
SEQ = 2048
D = 64
NCORES = 8
HPC = HEADS // NCORES          # heads per core = 2
LD = HPC * D                   # local head dim = 128
TSEQ = B * SEQ                 # 4096
CC = EMB // 128                # contraction chunks = 8
SCALE = float(EMB) ** -0.5     # 1/32

QCH = 512                      # q chunk (free dim of S^T matmuls)
NQ = SEQ // QCH                # 4 q-chunks per batch
NK = SEQ // 128                # 16 k-chunks per batch


def _build():
    nc = bacc.Bacc("TRN2", target_bir_lowering=False, debug=False,
                   num_devices=NCORES)

    xT = nc.dram_tensor("xT", [CC, 128, TSEQ], BF16, kind="ExternalInput")
    wqkv = nc.dram_tensor("wqkv", [128, CC * 3 * LD], BF16,
                          kind="ExternalInput")
    bqkv = nc.dram_tensor("bqkv", [LD, 3], F32, kind="ExternalInput")
    wout = nc.dram_tensor("wout", [LD, EMB], BF16, kind="ExternalInput")
    ones = nc.dram_tensor("ones", [1, D], BF16, kind="ExternalInput")
    onescol = nc.dram_tensor("onescol", [128, 1], BF16, kind="ExternalInput")
    y = nc.dram_tensor("y", [TSEQ // 128, 128, EMB], BF16,
                       kind="ExternalOutput")

    xT_c = xT.ap()
    wqkv_c = wqkv.ap()
    wchunk = 3 * LD

    with tile.TileContext(nc) as tc:
        with (
            tc.tile_pool(name="persist", bufs=1) as persist,
            tc.tile_pool(name="vt", bufs=2) as vtp,
            tc.tile_pool(name="psb", bufs=6) as psb,
            tc.tile_pool(name="norm", bufs=2) as normp,
            tc.tile_pool(name="yout", bufs=10) as youtp,
            tc.tile_pool(name="ps_st", bufs=2, space="PSUM") as ps_st,
            tc.tile_pool(name="ps_pv", bufs=1, space="PSUM") as ps_pv,
            tc.tile_pool(name="ps_misc", bufs=2, space="PSUM") as ps_misc,
        ):
            # ---- tiles that DMAs land in ----
            xfull = {}
            for kc in range(CC):
                for w in range(TSEQ // 512):
                    xfull[kc, w] = persist.tile([128, 512], BF16,
                                                tag=f"xf{kc}_{w}",
                                                name=f"xf{kc}_{w}")
            wall = persist.tile([128, CC * wchunk], BF16, tag="wall")
            bqkv_sb = persist.tile([LD, 3], F32, tag="bqkv")
            ones_sb = persist.tile([1, D], BF16, tag="ones")
            onescol_sb = persist.tile([128, 1], BF16, tag="onescol")
            wout_sb = persist.tile([LD, EMB], BF16, tag="wout")

            # ---- identity + PE warmup (HAM pre-warm during DMA wait) ----
            ident = persist.tile([128, 128], BF16, tag="ident")
            make_identity(nc, ident[:])
            for _ in range(16):
                pw = ps_misc.tile([128, 128], F32, tag="misc")
                nc.tensor.matmul(pw[:], ident[:], ident[:],
                                 start=True, stop=True)

            # ---- startup DMA dispatch ----
            # queue k handles (wall[k], x[k, w0]) so every contraction
            # chunk's pair lands together.  Only SP/Activation/gpsimd can
            # issue DMAs -> three queues.
            queues = [nc.scalar, nc.sync, nc.gpsimd]

            def dma_wall(kc, eng):
                eng.dma_start(wall[:, kc * wchunk:(kc + 1) * wchunk],
                              wqkv_c[:, kc * wchunk:(kc + 1) * wchunk])

            def dma_x(kc, w, eng):
                eng.dma_start(xfull[kc, w][:],
                              xT_c[kc, :, w * 512:(w + 1) * 512])

            nc.gpsimd.dma_start(bqkv_sb[:], bqkv.ap())
            for kc in range(CC):
                dma_wall(kc, queues[kc % 3])
                dma_x(kc, 0, queues[kc % 3])
            # small constants (needed from the first V unit / first norm)
            nc.sync.dma_start(ones_sb[:], ones.ap())
            nc.sync.dma_start(onescol_sb[:], onescol.ap())
            # QP zero-padding memsets on gpsimd (vector stays free for the
            # first bias-adds; gpsimd's DMA dispatches above are done)
            # (QP tiles declared below; memsets emitted after declaration)

            w_sb = {}
            for kc in range(CC):
                for i, nm in enumerate(("q", "k", "v")):
                    w_sb[nm, kc] = wall[:, (kc * 3 + i) * LD:
                                        (kc * 3 + i + 1) * LD]
            bias_sb = {nm: bqkv_sb[:, i:i + 1]
                       for i, nm in enumerate(("q", "k", "v"))}

            # persistent activations (per batch)
            # QT: compact Q^T — rows [h*64,(h+1)*64) hold Q^T_h.  The S^T
            # matmuls are row-tiled (tile_position (0,0)/(64,0)): two K=64
            # matmuls run concurrently in the PE array, so no zero-padding
            # is needed (measured 231ns/pair vs 432ns padded).
            QT = [persist.tile([LD, SEQ], BF16, tag=f"QT{b}",
                               name=f"QT{b}") for b in range(B)]
            KT = [persist.tile([LD, SEQ], BF16, tag=f"KT{b}", name=f"KT{b}")
                  for b in range(B)]
            outT = [persist.tile([LD, SEQ], BF16, tag=f"outT{b}",
                                 name=f"outT{b}") for b in range(B)]
            vaug = {}  # (b, kc) -> [128, 2*(D+1)] tile
            for b in range(B):
                for kc in range(NK):
                    vaug[b, kc] = persist.tile([128, 2 * (D + 1)], BF16,
                                               tag=f"vaug{b}_{kc}",
                                               name=f"vaug{b}_{kc}")

            # remaining x windows: w1 on all three queues, w2.. on
            # sync/gpsimd (scalar must be free for the exp stream).
            for kc in range(CC):
                dma_x(kc, 1, queues[kc % 3])
            nc.scalar.dma_start(wout_sb[:], wout.ap())
            for w in (2, 3, 4, 5, 6, 7):
                for kc in range(CC):
                    dma_x(kc, w, nc.gpsimd if kc % 2 else nc.sync)

            def phase_a_units(b, scs):
                """QKV^T projection + V transpose for batch b, as a list of
                per-tensor emit closures (~2us of PE work each)."""
                units = []
                for sc in scs:
                    s0 = sc * 512            # batch-local seq offset
                    g0 = b * SEQ + s0        # global column in xT

                    def unit(nm, b=b, s0=s0, g0=g0, sc=sc):
                        ps = ps_misc.tile([128, 512], F32, tag="misc")
                        for kc in range(CC):
                            nc.tensor.matmul(
                                ps[:], w_sb[nm, kc],
                                xfull[kc, g0 // 512][:],
                                start=(kc == 0), stop=(kc == CC - 1))
                        if nm == "q":
                            nc.vector.tensor_scalar_add(
                                QT[b][:, s0:s0 + 512], ps[:], bias_sb["q"])
                        elif nm == "k":
                            nc.vector.tensor_scalar_add(
                                KT[b][:, s0:s0 + 512], ps[:], bias_sb["k"])
                        else:
                            vt = vtp.tile([128, 512], BF16, tag="vt")
                            nc.vector.tensor_scalar_add(vt[:], ps[:],
                                                        bias_sb["v"])
                            for j in range(4):
                                va = vaug[b, sc * 4 + j]
                                pst = ps_misc.tile([128, 128], BF16,
                                                   tag="misc")
                                nc.tensor.transpose(
                                    pst[:], vt[:, j * 128:(j + 1) * 128],
                                    ident[:])
                                nc.vector.tensor_copy(va[:, 0:D],
                                                      pst[:, 0:D])
                                nc.vector.tensor_copy(va[:, D + 1:2 * D + 1],
                                                      pst[:, D:2 * D])
                                nc.vector.tensor_copy(va[:, D:D + 1],
                                                      onescol_sb[:])
                                nc.vector.tensor_copy(
                                    va[:, 2 * D + 1:2 * D + 2], onescol_sb[:])

                    for nm in ("q", "k", "v"):
                        units.append(lambda nm=nm, u=unit: u(nm))
                return units

            pending = []   # deferred output-projection units (global)

            def proj_unit(b, sc, n, eng=None, copy_eng=None):
                rt = b * (SEQ // 128) + sc
                ps = ps_misc.tile([128, 512], F32, tag="misc")
                nc.tensor.matmul(
                    ps[:], outT[b][:, sc * 128:(sc + 1) * 128],
                    wout_sb[:, n * 512:(n + 1) * 512],
                    start=True, stop=True)
                yt = youtp.tile([128, 512], BF16, tag="yt")
                if copy_eng is nc.scalar:
                    nc.scalar.copy(yt[:], ps[:])
                else:
                    nc.vector.tensor_copy(yt[:], ps[:])
                if eng is None:
                    eng = nc.gpsimd if (sc + n) % 2 else nc.sync
                eng.dma_start(
                    y.ap()[rt, :, n * 512:(n + 1) * 512], yt[:])

            def phase_bc(b, fill_units, pre=None, pop_slots=(), pops_per_slot=1,
                         fill_slots=()):
                """Attention for batch b; fill_units and deferred projection
                pops are injected inside the kc loop so the static per-engine
                order keeps both PE and ACT fed. `pre` maps kc -> producer
                units emitted before that kc group of q-chunk 0."""
                fill = list(fill_units)
                fi = 0
                pre = pre or {}

                def st_exp(q, kc):
                    """S^T pair (row-tiled, concurrent) + exp; returns pt."""
                    q0 = q * QCH
                    st = ps_st.tile([128, 2 * QCH], F32, tag="st")
                    k0 = kc * 128
                    for h in range(HPC):
                        nc.tensor.matmul(
                            st[:, h * QCH:(h + 1) * QCH],
                            KT[b][h * D:(h + 1) * D, k0:k0 + 128],
                            QT[b][h * D:(h + 1) * D, q0:q0 + QCH],
                            start=True, stop=True,
                            tile_position=(h * D, 0))
                    pt = psb.tile([128, 2 * QCH], BF16, tag="pt")
                    nc.scalar.activation(pt[:], st[:],
                                         mybir.ActivationFunctionType.Exp,
                                         scale=SCALE)
                    return pt

                pre_pts = {}
                for q in range(NQ):
                    q0 = q * QCH
                    pvs = [ps_pv.tile([D + 1, QCH], F32, tag=f"pv{h}",
                                      name=f"pv{h}") for h in range(HPC)]
                    for kc in range(NK):
                        if q == 0:
                            for u in pre.get(kc, ()):
                                u()
                        pt = pre_pts.pop((q, kc), None)
                        if pt is None:
                            pt = st_exp(q, kc)
                        for h in range(HPC):
                            nc.tensor.matmul(
                                pvs[h][:],
                                vaug[b, kc][:, h * (D + 1):(h + 1) * (D + 1)],
                                pt[:, h * QCH:(h + 1) * QCH],
                                start=(kc == 0), stop=(kc == NK - 1))
                        if kc in pop_slots:
                            for _ in range(pops_per_slot):
                                if pending:
                                    pending.pop(0)()
                        if q > 0 and kc in fill_slots and fi < len(fill):
                            fill[fi]()
                            fi += 1
                    # pre-issue the next q-chunk's first S^T/exp groups so
                    # ScalarE stays fed across the norm+projection boundary
                    if q + 1 < NQ:
                        for kc in (0, 1):
                            pre_pts[q + 1, kc] = st_exp(q + 1, kc)
                    # normalize: out^T[d, q] / colsum -> outT (bf16).
                    # evict each head's PV PSUM bank first (frees it for the
                    # next q-chunk), then broadcast + reciprocal + multiply.
                    nb, sb = [], []
                    for h in range(HPC):
                        nbt = normp.tile([D, QCH], BF16, tag=f"nb{h}",
                                         name=f"nb{h}")
                        sbt = normp.tile([1, QCH], BF16, tag=f"sb{h}",
                                         name=f"sb{h}")
                        nc.vector.tensor_copy(nbt[:], pvs[h][0:D, :])
                        nc.vector.tensor_copy(sbt[:], pvs[h][D:D + 1, :])
                        nb.append(nbt)
                        sb.append(sbt)
                    for h in range(HPC):
                        bc = ps_misc.tile([D, QCH], F32, tag="misc")
                        nc.tensor.matmul(bc[:], ones_sb[:], sb[h][:],
                                         start=True, stop=True)
                        rc = normp.tile([D, QCH], F32, tag="rc")
                        nc.vector.reciprocal_approx_fast(rc[:], bc[:])
                        nc.vector.tensor_mul(
                            outT[b][h * D:(h + 1) * D, q0:q0 + QCH],
                            nb[h][:], rc[:])
                    pending.extend(
                        (lambda b=b, sc=sc, n=n, eng=None, copy_eng=None:
                         proj_unit(b, sc, n, eng, copy_eng))
                        for sc in range(4 * q, 4 * q + 4)
                        for n in range(EMB // 512))
                while fi < len(fill):
                    fill[fi]()
                    fi += 1

            for u in phase_a_units(0, [0]):
                u()
            # b0: pop deferred projections sparingly (keep work for b1,
            # whose loop has no QKV fills); b1: double ration.
            phase_bc(0, phase_a_units(1, range(4)),
                     pre={4 * s: phase_a_units(0, [s]) for s in (1, 2, 3)},
                     pop_slots=(1, 5, 9, 13), pops_per_slot=1,
                     fill_slots=(4, 7, 10, 13, 15))
            phase_bc(1, [], pop_slots=(1, 3, 5, 7, 9, 11, 13, 15),
                     pops_per_slot=2)
            # tail flush: remaining projections with copies and DMAs spread
            # across engines (scalar is idle once the last exp retires)
            engs = [nc.scalar, nc.sync, nc.gpsimd]
            copies = [nc.vector, nc.scalar]
            for j, p in enumerate(pending):
                p(eng=engs[j % 3], copy_eng=copies[j % 2])
            pending.clear()

    nc.compile()
    return nc


_NC = None


def _get_nc():
    global _NC
    if _NC is None:
        _NC = _build()
    return _NC


def kernel(x, W_qkv, b_qkv, W_out, b_out):
    x = np.asarray(x, dtype=np.float32)
    W_qkv = np.asarray(W_qkv, dtype=np.float32)
    b_qkv = np.asarray(b_qkv, dtype=np.float32)
    W_out = np.asarray(W_out, dtype=np.float32)
    b_out = np.asarray(b_out, dtype=np.float32)

    nc = _get_nc()

    xT = np.ascontiguousarray(
        x.reshape(TSEQ, EMB).T.astype(NPBF16)).reshape(CC, 128, TSEQ)
    Wr = W_qkv.reshape(EMB, 3, HEADS, D)
    br = b_qkv.reshape(3, HEADS, D)
    ones = np.ones((1, D), dtype=NPBF16)
    onescol = np.ones((128, 1), dtype=NPBF16)

    in_maps = []
    for c in range(NCORES):
        h0, h1 = HPC * c, HPC * (c + 1)
        in_maps.append({
            "xT": xT,
            "wqkv": np.ascontiguousarray(
                np.stack([Wr[:, i, h0:h1].reshape(CC, 128, LD)
                          for i in range(3)], axis=1)
                .transpose(2, 0, 1, 3).reshape(128, CC * 3 * LD)
            ).astype(NPBF16),
            "bqkv": np.ascontiguousarray(
                np.stack([br[i, h0:h1].reshape(LD) for i in range(3)],
                         axis=1)),
            "wout": np.ascontiguousarray(
                W_out[LD * c:LD * (c + 1)]).astype(NPBF16),
            "ones": ones,
            "onescol": onescol,
        })

    res = bass_utils.run_bass_kernel_spmd(
        nc, in_maps, core_ids=list(range(NCORES)), trace=False)

    acc = np.zeros((TSEQ // 128, 128, EMB), dtype=np.float64)
    for c in range(NCORES):
        acc += res.results[c]["y"].astype(np.float64)
    out = (acc.reshape(TSEQ, EMB) + b_out).astype(np.float32)
    return out.reshape(B, SEQ, EMB)


# revision 18
# speedup vs baseline: 1.1720x; 1.1720x over previous
"""Multi-head attention (B=2, N=2048, C=1024, H=16) on 8 trn2 NeuronCores.

Tensor-parallel over heads: core c computes heads {2c, 2c+1} for both batch
elements and emits a partial output y_c = attn_out_c @ W_out[local rows];
the host sums the 8 partials (bf16) and adds b_out.

Design (single TileContext, fully unrolled; emission order IS the static
per-engine program order):

  - The attention inner loop is ACT-bound: exp of S^T [128,1024] costs
    (1024+352)/1.2 = 1147ns per k-chunk vs ~660ns of PE work (S^T pair
    231ns row-tiled + PV pair 432ns).  Everything else (QKV projection,
    output projection, V transposes) is cut into ~1us closures and
    injected one-per-k-chunk into the exp shadow, with just-in-time
    deadlines (KT/vaug for seq-block sc must land before k-chunk 4*sc of
    the first q-chunk).
  - S^T = K @ Q^T as two concurrent row-tiled K=64 matmuls
    (tile_position (0,0)/(64,0); measured 231ns/pair vs 432ns padded).
  - PV via ones-augmented V (65th stationary column accumulates softmax
    denominators).  V transposed on the PE.
  - exp pre-issue crosses chunk boundaries (next chunk's k-chunks 0/1 are
    issued at k-chunks 14/15 of the current chunk, including across the
    batch boundary) so ScalarE never drains at norm time.
  - Normalization: per head evict [64,512]+[1,512] to bf16 (releases the
    PV PSUM bank fast), ones-broadcast bf16 matmul, reciprocal, multiply
    -> outT bf16.
  - Output projection in bf16 (fp32r matmuls measured at HALF rate);
    y partials stored/DMAd as bf16 (halves the 16MB output traffic).
  - Startup: wall chunk kc + x window-0 chunk kc dispatched as pairs
    round-robin over the three DMA queues (SP/Activation/gpsimd) so the
    first QKV matmul starts ~10us in; identity-matmul warmup keeps the
    PE HAM clock-gate warm through the DMA wait.

Measured rel-err ~5.6e-3 of output scale vs the fp32 reference.
"""
import sys

sys.path.insert(0, "/opt/trn_rl_repo")

import ml_dtypes
import numpy as np

import concourse.bacc as bacc
import concourse.mybir as mybir
import concourse.tile as tile
from concourse import bass_utils
from concourse.masks import make_identity

F32 = mybir.dt.float32
BF16 = mybir.dt.bfloat16
NPBF16 = ml_dtypes.bfloat16

EMB = 1024
HEADS = 16
B = 2
SEQ = 2048
D = 64
NCORES = 8
HPC = HEADS // NCORES          # heads per core = 2
LD = HPC * D                   # local head dim = 128
TSEQ = B * SEQ                 # 4096
CC = EMB // 128                # contraction chunks = 8
SCALE = float(EMB) ** -0.5     # 1/32

QCH = 512                      # q chunk (free dim of S^T matmuls)
NQ = SEQ // QCH                # 4 q-chunks per batch
NK = SEQ // 128                # 16 k-chunks per batch


def _build():
    nc = bacc.Bacc("TRN2", target_bir_lowering=False, debug=False,
                   num_devices=NCORES)

    xT = nc.dram_tensor("xT", [CC, 128, TSEQ], BF16, kind="ExternalInput")
    wqkv = nc.dram_tensor("wqkv", [128, CC * 3 * LD], BF16,
                          kind="ExternalInput")
    bqkv = nc.dram_tensor("bqkv", [LD, 3], F32, kind="ExternalInput")
    wout = nc.dram_tensor("wout", [LD, EMB], BF16, kind="ExternalInput")
    ones = nc.dram_tensor("ones", [1, D], BF16, kind="ExternalInput")
    onescol = nc.dram_tensor("onescol", [128, 1], BF16, kind="ExternalInput")
    y = nc.dram_tensor("y", [TSEQ // 128, 128, EMB], BF16,
                       kind="ExternalOutput")

    xT_c = xT.ap()
    wqkv_c = wqkv.ap()
    wchunk = 3 * LD

    with tile.TileContext(nc) as tc:
        with (
            tc.tile_pool(name="persist", bufs=1) as persist,
            tc.tile_pool(name="vt", bufs=2) as vtp,
            tc.tile_pool(name="psb", bufs=6) as psb,
            tc.tile_pool(name="norm", bufs=2) as normp,
            tc.tile_pool(name="yout", bufs=10) as youtp,
            tc.tile_pool(name="ps_st", bufs=2, space="PSUM") as ps_st,
            tc.tile_pool(name="ps_pv", bufs=1, space="PSUM") as ps_pv,
            tc.tile_pool(name="ps_misc", bufs=2, space="PSUM") as ps_misc,
        ):
            # ---- tiles that DMAs land in ----
            xfull = {}
            for kc in range(CC):
                for w in range(TSEQ // 512):
                    xfull[kc, w] = persist.tile([128, 512], BF16,
                                                tag=f"xf{kc}_{w}",
                                                name=f"xf{kc}_{w}")
            wall = persist.tile([128, CC * wchunk], BF16, tag="wall")
            bqkv_sb = persist.tile([LD, 3], F32, tag="bqkv")
            ones_sb = persist.tile([1, D], BF16, tag="ones")
            onescol_sb = persist.tile([128, 1], BF16, tag="onescol")
            wout_sb = persist.tile([LD, EMB], BF16, tag="wout")

            # ---- identity + PE warmup (HAM pre-warm during DMA wait) ----
            ident = persist.tile([128, 128], BF16, tag="ident")
            make_identity(nc, ident[:])
            for _ in range(16):
                pw = ps_misc.tile([128, 128], F32, tag="misc")
                nc.tensor.matmul(pw[:], ident[:], ident[:],
                                 start=True, stop=True)

            # ---- startup DMA dispatch (3 queues: Activation/SP/gpsimd) ----
            queues = [nc.scalar, nc.sync, nc.gpsimd]

            def dma_wall(kc, eng):
                eng.dma_start(wall[:, kc * wchunk:(kc + 1) * wchunk],
                              wqkv_c[:, kc * wchunk:(kc + 1) * wchunk])

            def dma_x(kc, w, eng):
                eng.dma_start(xfull[kc, w][:],
                              xT_c[kc, :, w * 512:(w + 1) * 512])

            nc.gpsimd.dma_start(bqkv_sb[:], bqkv.ap())
            for kc in range(CC):
                dma_wall(kc, queues[kc % 3])
                dma_x(kc, 0, queues[kc % 3])
            nc.sync.dma_start(ones_sb[:], ones.ap())
            nc.sync.dma_start(onescol_sb[:], onescol.ap())

            w_sb = {}
            for kc in range(CC):
                for i, nm in enumerate(("q", "k", "v")):
                    w_sb[nm, kc] = wall[:, (kc * 3 + i) * LD:
                                        (kc * 3 + i + 1) * LD]
            bias_sb = {nm: bqkv_sb[:, i:i + 1]
                       for i, nm in enumerate(("q", "k", "v"))}

            # persistent activations (per batch).  QT is compact: rows
            # [h*64,(h+1)*64) hold Q^T_h (row-tiled S^T needs no padding).
            QT = [persist.tile([LD, SEQ], BF16, tag=f"QT{b}",
                               name=f"QT{b}") for b in range(B)]
            KT = [persist.tile([LD, SEQ], BF16, tag=f"KT{b}", name=f"KT{b}")
                  for b in range(B)]
            outT = [persist.tile([LD, SEQ], BF16, tag=f"outT{b}",
                                 name=f"outT{b}") for b in range(B)]
            vaug = {}  # (b, kc) -> [128, 2*(D+1)] tile
            for b in range(B):
                for kc in range(NK):
                    vaug[b, kc] = persist.tile([128, 2 * (D + 1)], BF16,
                                               tag=f"vaug{b}_{kc}",
                                               name=f"vaug{b}_{kc}")

            # remaining x windows: w1 on all three queues, w2.. on
            # sync/gpsimd (scalar must be free for the exp stream).
            for kc in range(CC):
                dma_x(kc, 1, queues[kc % 3])
            nc.scalar.dma_start(wout_sb[:], wout.ap())
            for w in (2, 3, 4, 5, 6, 7):
                for kc in range(CC):
                    dma_x(kc, w, nc.gpsimd if kc % 2 else nc.sync)

            # ---- QKV projection closures (~1us of PE work each) ----
            def qkv_closures(b, sc, nm):
                """List of closures computing tensor nm for seq-block sc of
                batch b: [first 4 accum MMs], [last 4 + eviction], and for
                v two transpose closures (2 x 128-cols each)."""
                s0 = sc * 512
                w = (b * SEQ + s0) // 512
                state = {}

                def mm_a():
                    ps = ps_misc.tile([128, 512], F32, tag="misc")
                    state["ps"] = ps
                    for kc in range(4):
                        nc.tensor.matmul(ps[:], w_sb[nm, kc],
                                         xfull[kc, w][:],
                                         start=(kc == 0), stop=False)

                def mm_b():
                    ps = state["ps"]
                    for kc in range(4, CC):
                        nc.tensor.matmul(ps[:], w_sb[nm, kc],
                                         xfull[kc, w][:],
                                         start=False, stop=(kc == CC - 1))
                    if nm == "q":
                        nc.vector.tensor_scalar_add(
                            QT[b][:, s0:s0 + 512], ps[:], bias_sb["q"])
                    elif nm == "k":
                        nc.vector.tensor_scalar_add(
                            KT[b][:, s0:s0 + 512], ps[:], bias_sb["k"])
                    else:
                        vt = vtp.tile([128, 512], BF16, tag="vt")
                        nc.vector.tensor_scalar_add(vt[:], ps[:],
                                                    bias_sb["v"])
                        state["vt"] = vt

                def transp(j0):
                    def f():
                        vt = state["vt"]
                        for j in (j0, j0 + 1):
                            va = vaug[b, sc * 4 + j]
                            pst = ps_misc.tile([128, 128], BF16, tag="misc")
                            nc.tensor.transpose(
                                pst[:], vt[:, j * 128:(j + 1) * 128],
                                ident[:])
                            nc.vector.tensor_copy(va[:, 0:D], pst[:, 0:D])
                            nc.vector.tensor_copy(va[:, D + 1:2 * D + 1],
                                                  pst[:, D:2 * D])
                            nc.vector.tensor_copy(va[:, D:D + 1],
                                                  onescol_sb[:])
                            nc.vector.tensor_copy(
                                va[:, 2 * D + 1:2 * D + 2], onescol_sb[:])
                    return f

                cls = [mm_a, mm_b]
                if nm == "v":
                    cls += [transp(0), transp(2)]
                return cls

            pending = []   # deferred output-projection units (global)

            def proj_unit(b, sc, n, eng=None, copy_eng=None):
                rt = b * (SEQ // 128) + sc
                ps = ps_misc.tile([128, 512], F32, tag="misc")
                nc.tensor.matmul(
                    ps[:], outT[b][:, sc * 128:(sc + 1) * 128],
                    wout_sb[:, n * 512:(n + 1) * 512],
                    start=True, stop=True)
                yt = youtp.tile([128, 512], BF16, tag="yt")
                if copy_eng is nc.scalar:
                    nc.scalar.copy(yt[:], ps[:])
                else:
                    nc.vector.tensor_copy(yt[:], ps[:])
                if eng is None:
                    eng = nc.gpsimd if (sc + n) % 2 else nc.sync
                eng.dma_start(
                    y.ap()[rt, :, n * 512:(n + 1) * 512], yt[:])

            pre_pts = {}   # (b, q, kc) -> pt tile (exp pre-issued)

            def st_exp(b, q, kc):
                """S^T pair (row-tiled, concurrent) + exp; returns pt."""
                q0 = q * QCH
                st = ps_st.tile([128, 2 * QCH], F32, tag="st")
                k0 = kc * 128
                for h in range(HPC):
                    nc.tensor.matmul(
                        st[:, h * QCH:(h + 1) * QCH],
                        KT[b][h * D:(h + 1) * D, k0:k0 + 128],
                        QT[b][h * D:(h + 1) * D, q0:q0 + QCH],
                        start=True, stop=True,
                        tile_position=(h * D, 0))
                pt = psb.tile([128, 2 * QCH], BF16, tag="pt")
                nc.scalar.activation(pt[:], st[:],
                                     mybir.ActivationFunctionType.Exp,
                                     scale=SCALE)
                return pt

            def chunk(b, q, inj, pops, nxt):
                """One attention q-chunk.  inj: kc -> injected closures.
                pops: kc -> number of deferred projections to emit.
                nxt: (b, q) of the next chunk (its k-chunks 0/1 are
                exp-pre-issued at k-chunks 14/15) or None."""
                q0 = q * QCH
                pvs = [ps_pv.tile([D + 1, QCH], F32, tag=f"pv{h}",
                                  name=f"pv{h}") for h in range(HPC)]
                for kc in range(NK):
                    for u in inj.get(kc, ()):
                        u()
                    pt = pre_pts.pop((b, q, kc), None)
                    if pt is None:
                        pt = st_exp(b, q, kc)
                    if kc == NK - 1 and nxt is not None:
                        # keep ScalarE fed across the norm boundary: the
                        # next chunk's first two exps queue right behind
                        # this chunk's last one
                        pre_pts[nxt + (0,)] = st_exp(nxt[0], nxt[1], 0)
                        pre_pts[nxt + (1,)] = st_exp(nxt[0], nxt[1], 1)
                    for h in range(HPC):
                        nc.tensor.matmul(
                            pvs[h][:],
                            vaug[b, kc][:, h * (D + 1):(h + 1) * (D + 1)],
                            pt[:, h * QCH:(h + 1) * QCH],
                            start=(kc == 0), stop=(kc == NK - 1))
                    for _ in range(pops.get(kc, 0)):
                        if pending:
                            pending.pop(0)()
                # normalize: evict both PV banks first (bf16), then
                # broadcast + reciprocal + multiply -> outT bf16.
                nb, sb = [], []
                for h in range(HPC):
                    nbt = normp.tile([D, QCH], BF16, tag=f"nb{h}",
                                     name=f"nb{h}")
                    sbt = normp.tile([1, QCH], BF16, tag=f"sb{h}",
                                     name=f"sb{h}")
                    nc.vector.tensor_copy(nbt[:], pvs[h][0:D, :])
                    nc.vector.tensor_copy(sbt[:], pvs[h][D:D + 1, :])
                    nb.append(nbt)
                    sb.append(sbt)
                for h in range(HPC):
                    bc = ps_misc.tile([D, QCH], F32, tag="misc")
                    nc.tensor.matmul(bc[:], ones_sb[:], sb[h][:],
                                     start=True, stop=True)
                    rc = normp.tile([D, QCH], F32, tag="rc")
                    nc.vector.reciprocal_approx_fast(rc[:], bc[:])
                    nc.vector.tensor_mul(
                        outT[b][h * D:(h + 1) * D, q0:q0 + QCH],
                        nb[h][:], rc[:])
                pending.extend(
                    (lambda b=b, sc=sc, n=n, eng=None, copy_eng=None:
                     proj_unit(b, sc, n, eng, copy_eng))
                    for sc in range(4 * q, 4 * q + 4)
                    for n in range(EMB // 512))

            # ---- pre-attention: QKV for batch-0 seq-block 0 ----
            for nm in ("q", "k", "v"):
                for cl in qkv_closures(0, 0, nm):
                    cl()

            # ---- chunk schedule ----
            # shorthand: c(b, sc, nm) -> closure list
            c = qkv_closures
            k1 = c(0, 1, "k"); v1 = c(0, 1, "v"); q1 = c(0, 1, "q")
            k2 = c(0, 2, "k"); v2 = c(0, 2, "v"); q2 = c(0, 2, "q")
            k3 = c(0, 3, "k"); v3 = c(0, 3, "v"); q3 = c(0, 3, "q")
            K0 = c(1, 0, "k"); V0 = c(1, 0, "v"); Q0 = c(1, 0, "q")
            K1 = c(1, 1, "k"); V1 = c(1, 1, "v"); Q1 = c(1, 1, "q")
            K2 = c(1, 2, "k"); V2 = c(1, 2, "v"); Q2 = c(1, 2, "q")
            K3 = c(1, 3, "k"); V3 = c(1, 3, "v"); Q3 = c(1, 3, "q")

            inj00 = {0: [k1[0]], 1: [k1[1]], 2: [v1[0]], 3: [v1[1]],
                     4: [v1[2], k2[0]], 5: [v1[3], k2[1]],
                     6: [v2[0]], 7: [v2[1]],
                     8: [v2[2], k3[0]], 9: [v2[3], k3[1]],
                     10: [v3[0]], 11: [v3[1]],
                     12: [v3[2], q1[0]], 13: [v3[3], q1[1]]}
            inj01 = {0: [q2[0]], 1: [q2[1]], 2: [q3[0]], 3: [q3[1]],
                     4: [K0[0]], 5: [K0[1]], 6: [V0[0]], 7: [V0[1]],
                     8: [V0[2]], 9: [V0[3]], 10: [K1[0]], 11: [K1[1]],
                     12: [V1[0]], 13: [V1[1]], 14: [V1[2]], 15: [V1[3]]}
            inj02 = {0: [K2[0]], 1: [K2[1]], 2: [V2[0]], 3: [V2[1]],
                     4: [V2[2]], 5: [V2[3]], 6: [K3[0]], 7: [K3[1]],
                     8: [V3[0]], 9: [V3[1]], 10: [V3[2]], 11: [V3[3]],
                     12: [Q0[0]], 13: [Q0[1]], 14: [Q1[0]], 15: [Q1[1]]}
            inj03 = {0: [Q2[0]], 1: [Q2[1]], 2: [Q3[0]], 3: [Q3[1]]}

            p0 = {}                                      # no pops
            p1 = {9: 1, 13: 1}
            p3 = {5: 1, 7: 1, 9: 1, 11: 1, 13: 1, 15: 1}
            pb1 = {kc: 2 for kc in range(1, NK, 2)}      # batch-1 ration

            seq = [
                ((0, 0), inj00, p0),
                ((0, 1), inj01, p1),
                ((0, 2), inj02, p0),
                ((0, 3), inj03, p3),
                ((1, 0), {}, pb1),
                ((1, 1), {}, pb1),
                ((1, 2), {}, pb1),
                ((1, 3), {}, pb1),
            ]
            for i, ((b, q), inj, pops) in enumerate(seq):
                nxt = seq[i + 1][0] if i + 1 < len(seq) else None
                chunk(b, q, inj, pops, nxt)

            # tail flush: remaining projections with copies and DMAs spread
            # across engines (scalar is idle once the last exp retires)
            engs = [nc.scalar, nc.sync, nc.gpsimd]
            copies = [nc.vector, nc.scalar]
            for j, p in enumerate(pending):
                p(eng=engs[j % 3], copy_eng=copies[j % 2])
            pending.clear()

    nc.compile()
    return nc


_NC = None


def _get_nc():
    global _NC
    if _NC is None:
        _NC = _build()
    return _NC


def kernel(x, W_qkv, b_qkv, W_out, b_out):
    x = np.asarray(x, dtype=np.float32)
    W_qkv = np.asarray(W_qkv, dtype=np.float32)
    b_qkv = np.asarray(b_qkv, dtype=np.float32)
    W_out = np.asarray(W_out, dtype=np.float32)
    b_out = np.asarray(b_out, dtype=np.float32)

    nc = _get_nc()

    xT = np.ascontiguousarray(
        x.reshape(TSEQ, EMB).T.astype(NPBF16)).reshape(CC, 128, TSEQ)
    Wr = W_qkv.reshape(EMB, 3, HEADS, D)
    br = b_qkv.reshape(3, HEADS, D)
    ones = np.ones((1, D), dtype=NPBF16)
    onescol = np.ones((128, 1), dtype=NPBF16)

    in_maps = []
    for c in range(NCORES):
        h0, h1 = HPC * c, HPC * (c + 1)
        in_maps.append({
            "xT": xT,
            "wqkv": np.ascontiguousarray(
                np.stack([Wr[:, i, h0:h1].reshape(CC, 128, LD)
                          for i in range(3)], axis=1)
                .transpose(2, 0, 1, 3).reshape(128, CC * 3 * LD)
            ).astype(NPBF16),
            "bqkv": np.ascontiguousarray(
                np.stack([br[i, h0:h1].reshape(LD) for i in range(3)],
                         axis=1)),
            "wout": np.ascontiguousarray(
                W_out[LD * c:LD * (c + 1)]).astype(NPBF16),
            "ones": ones,
            "onescol": onescol,
        })

    res = bass_utils.run_bass_kernel_spmd(
        nc, in_maps, core_ids=list(range(NCORES)), trace=False)

    acc = np.zeros((TSEQ // 128, 128, EMB), dtype=np.float64)
    for c in range(NCORES):
        acc += res.results[c]["y"].astype(np.float64)
    out = (acc.reshape(TSEQ, EMB) + b_out).astype(np.float32)
    return out.reshape(B, SEQ, EMB)
